# revision 9
# baseline (speedup 1.0000x reference)
"""Trainium2 Bass kernel for nn_CustomConv1D_d (rank-1 dense conv1d, stride 21).

Math: out[b, t, o] = r[b, t] for all o in [0, 237), where
  r[b, t] = sum_k w[k] * sum_c x[b, 21 t + k, c],  w = softmax(p3*i + p4*i^2).

Strategy (pure data parallel over batch, 4 batches per core):
  - Per core, view x flat as [2048 groups, 4977]; partition p owns a
    CONTIGUOUS run of cnt[p] groups (prefix layout).  cnt[p] is 13 for the
    8 partitions served by SDMA engine 15 (92-95, 124-127 -- that engine
    runs ~20% slower because the dynamic-queue descriptor rings share its
    AXI port), 17 for partitions [0, 24), 16 elsewhere.  This balances
    per-engine DMA finish times.
  - Input DMA in phases: one dma_start per (phase, class-run) moving
    w consecutive groups per partition = w*19908 contiguous bytes per
    descriptor.  Ring of 2 phase buffers overlaps DMA with DVE.
  - DVE: per step, segmented reduce over channels -> per-tap sums
    skg[:, s*21:(s+1)*21]; per phase one tensor_mul by tap weights + one
    reduce over taps -> acc[:, s] = r values.
  - ACT broadcasts r across 237 channels into chunk buffers; output DMA
    per (chunk, class-run): per-partition w*948B contiguous descriptors.
"""

import numpy as np
from contextlib import ExitStack

import concourse.bass as bass
import concourse.tile as tile
import concourse.mybir as mybir
from concourse.bass_utils import run_bass_kernel_spmd

TAPS = 21
C = 237
B = 32
L = 10752
T = 512
NCORES = 8
BPC = B // NCORES            # 4 batches per core
NG = BPC * T                 # 2048 groups per core
FD = TAPS * C                # 4977 elements per group
F32 = mybir.dt.float32

# Partition classes: (partition range, groups per partition).
# prefix-sum start: [0,24):17p ; [24,92):408+16(p-24) ; [92,96):1496+13(p-92)
# [96,124):1548+16(p-96) ; [124,128):1996+13(p-124)  -> total 2048.
CLASSES = [(0, 24, 17), (24, 92, 16), (92, 96, 13), (96, 124, 16), (124, 128, 13)]
STARTS = {}
_g = 0
for _a, _b, _c in CLASSES:
    for _p in range(_a, _b):
        STARTS[_p] = _g
        _g += _c
assert _g == NG, _g

MAXW = 3                      # widest phase (groups per partition per phase)

# Input phases: (step_lo, width, runs) where runs are (part_lo, part_hi, cls).
# Step s valid for class cls iff s < cls.  For steps 13..15 the [92,96)
# partitions are covered by extending the [24,92) run (slope 16 matches the
# prefix at p=92); they load a neighbour's group -- harmless, never read out.
# the [124,128) run is split so the rearrange slice stays in-bounds (the
# formal [g0, g0+cls*n) window of a joint run would pass row 2048)
RUN_ALL = [
    (0, 24, 17),
    (24, 92, 16),
    (92, 96, 13),
    (96, 124, 16),
    (124, 127, 13),
    (127, 128, 13),
]
RUN_AB = [(0, 24, 17), (24, 96, 16), (96, 124, 16)]   # steps 13..15 (ext trick)
RUN_A = [(0, 24, 17)]                                  # step 16
PHASES = [
    (16, 1, RUN_A, 24),
    (0, 1, RUN_ALL, 128),
    (1, 2, RUN_ALL, 128),
    (3, 3, RUN_ALL, 128),
    (6, 3, RUN_ALL, 128),
    (9, 3, RUN_ALL, 128),
    (12, 1, RUN_ALL, 128),
    (13, 2, RUN_AB, 124),
    (15, 1, RUN_AB, 124),
]

# Output chunks: (step_lo, step_hi, runs, active partition bound)
# NB: output runs must be the REAL class runs (no [24,96) extension --
# that trick is only sound for redundant input loads, never for stores).
OCHUNKS = [
    (0, 6, RUN_ALL, 128),
    (6, 13, RUN_ALL, 128),
    (13, 16, [(0, 24, 17), (24, 92, 16), (96, 124, 16)], 124),
    (16, 17, RUN_A, 24),
]
OMAXW = 7


class _TileContext(tile.TileContext):
    """TileContext with a post-scheduling pass that splits instructions
    carrying >1 sem wait onto preceding single-wait nops on the same
    engine -- the pinned neuronxcc rejects instructions with multiple
    sync wait commands."""

    def schedule_and_allocate(self):
        ret = super().schedule_and_allocate()
        self._split_multi_waits()
        return ret

    def _split_multi_waits(self):
        nc = self.nc
        for fn in nc.m.functions:
            for bb in fn.blocks:
                if not any(
                    inst.sync_info
                    and inst.sync_info.on_wait
                    and len(inst.sync_info.on_wait) > 1
                    for inst in bb.instructions
                ):
                    continue
                new_insts = []
                for inst in bb.instructions:
                    si = inst.sync_info
                    waits = list(si.on_wait) if si and si.on_wait else []
                    if len(waits) > 1:
                        si.on_wait = waits[-1:]
                        for w in waits[:-1]:
                            nop = mybir.InstNoOp(
                                name=f"I-splitw-{nc.next_id()}",
                                engine=inst.engine,
                                sync_info=mybir.SyncInfo(on_wait=[w], on_update=[]),
                            )
                            nc.register_instruction(nop, overwrite=True)
                            new_insts.append(nop)
                    new_insts.append(inst)
                bb.instructions[:] = new_insts


def _strided(t, g0, n, cls, w, fsz):
    """AP [n partitions, w*fsz] over rows [g0 + cls*p + j] of t (j < w)."""
    if n == 1:
        return t[g0 : g0 + w, :].rearrange("(p g) f -> p (g f)", g=w)
    return t[g0 : g0 + cls * n, :].rearrange("(p g) f -> p (g f)", g=cls)[
        :, : w * fsz
    ]


def _in_ap(x, part_lo, part_hi, cls, s, w):
    """Input AP: partitions [part_lo, part_hi), groups start_p+s .. +s+w."""
    return _strided(x, STARTS[part_lo] + s, part_hi - part_lo, cls, w, FD)


def _out_ap(y, part_lo, part_hi, cls, s, w):
    return _strided(y, STARTS[part_lo] + s, part_hi - part_lo, cls, w, C)


def _build():
    nc = bass.Bass("TRN2", target_bir_lowering=False, debug=False)
    x = nc.dram_tensor("x", [NG, FD], F32, kind="ExternalInput").ap()
    wv = nc.dram_tensor("wv", [17 * TAPS], F32, kind="ExternalInput").ap()
    y = nc.dram_tensor("y", [NG, C], F32, kind="ExternalOutput").ap()

    with _TileContext(nc) as tc:
        with ExitStack() as ctx:
            xin = ctx.enter_context(tc.tile_pool(name="xin", bufs=2))
            sp = ctx.enter_context(tc.tile_pool(name="sp", bufs=1))
            op = ctx.enter_context(tc.tile_pool(name="op", bufs=2))

            wrep = sp.tile([128, 17 * TAPS], F32)
            nc.gpsimd.dma_start(wrep[:], wv[None, :].broadcast_to([128, 17 * TAPS]))
            skg = sp.tile([128, 17 * TAPS], F32)
            skw = sp.tile([128, MAXW * TAPS], F32)
            acc = sp.tile([128, 17], F32)

            bcast_done = set()
            steps_done = set()

            def do_bcast_chunk(c_lo, c_hi, runs, pbound, osb):
                for j, s in enumerate(range(c_lo, c_hi)):
                    nc.scalar.activation(
                        osb[:pbound, j * C : (j + 1) * C],
                        acc[:pbound, s : s + 1].broadcast_to([pbound, C]),
                        mybir.ActivationFunctionType.Identity,
                    )
                for (a, b, cls) in runs:
                    w = min(c_hi, cls) - c_lo
                    nc.scalar.dma_start(
                        _out_ap(y, a, b, cls, c_lo, w),
                        osb[a:b, : w * C],
                    )

            for (s0, w, runs, pbound) in PHASES:
                xt = xin.tile([128, MAXW * FD], F32, tag="xt")
                if s0 == 15:
                    # tail phase: split taps so the final reduce is short
                    for (a, b, cls) in runs:
                        src = _in_ap(x, a, b, cls, s0, 1)
                        for k0, tk in ((0, 11), (11, 10)):
                            nc.sync.dma_start(
                                xt[a:b, k0 * C : (k0 + tk) * C],
                                src[:, k0 * C : (k0 + tk) * C],
                            )
                    for k0, tk in ((0, 11), (11, 10)):
                        nc.vector.reduce_sum(
                            skg[:pbound, s0 * TAPS + k0 : s0 * TAPS + k0 + tk],
                            xt[:pbound, k0 * C : (k0 + tk) * C].rearrange(
                                "p (k c) -> p k c", c=C
                            ),
                            axis=mybir.AxisListType.X,
                        )
                else:
                    for (a, b, cls) in runs:
                        nc.sync.dma_start(
                            xt[a:b, : w * FD], _in_ap(x, a, b, cls, s0, w)
                        )
                    for j in range(w):
                        nc.vector.reduce_sum(
                            skg[:pbound, (s0 + j) * TAPS : (s0 + j + 1) * TAPS],
                            xt[:pbound, j * FD : (j + 1) * FD].rearrange(
                                "p (k c) -> p k c", c=C
                            ),
                            axis=mybir.AxisListType.X,
                        )
                nc.vector.tensor_mul(
                    skw[:pbound, : w * TAPS],
                    skg[:pbound, s0 * TAPS : (s0 + w) * TAPS],
                    wrep[:pbound, s0 * TAPS : (s0 + w) * TAPS],
                )
                nc.vector.reduce_sum(
                    acc[:pbound, s0 : s0 + w],
                    skw[:pbound, : w * TAPS].rearrange("p (o k) -> p o k", k=TAPS),
                    axis=mybir.AxisListType.X,
                )
                # emit output chunks whose steps are now all complete
                steps_done.update(range(s0, s0 + w))
                for ci, (c_lo, c_hi, oruns, opb) in enumerate(OCHUNKS):
                    if ci in bcast_done:
                        continue
                    if all(s in steps_done for s in range(c_lo, c_hi)):
                        osb = op.tile([128, OMAXW * C], F32, tag="osb")
                        do_bcast_chunk(c_lo, c_hi, oruns, opb, osb)
                        bcast_done.add(ci)
    return nc


_NC_CACHE = {}


def _get_nc():
    if "nc" not in _NC_CACHE:
        _NC_CACHE["nc"] = _build()
    return _NC_CACHE["nc"]


def _tap_weights(param3: float, param4: float) -> np.ndarray:
    i = np.arange(1, TAPS + 1, dtype=np.float32)
    logits = (np.float32(param3) * i + np.float32(param4) * i * i).astype(np.float32)
    e = np.exp(logits - logits.max(), dtype=np.float32)
    w = (e / e.sum()).astype(np.float32)
    return np.tile(w, 17).astype(np.float32)  # [17*TAPS]


def run_with_results(inputs, **spmd_kwargs):
    x = np.ascontiguousarray(np.asarray(inputs["inputs"], dtype=np.float32))
    assert x.shape == (B, L, C), x.shape
    wv = _tap_weights(
        float(np.asarray(inputs["param3"])), float(np.asarray(inputs["param4"]))
    )
    xs = x.reshape(NCORES, NG, FD)
    in_maps = [{"x": xs[i], "wv": wv} for i in range(NCORES)]
    res = run_bass_kernel_spmd(_get_nc(), in_maps, list(range(NCORES)), **spmd_kwargs)
    out = np.stack([res.results[i]["y"] for i in range(NCORES)])
    return out.reshape(B, T, C).astype(np.float32, copy=False), res


def kernel(**inputs) -> np.ndarray:
    out, _ = run_with_results(inputs)
    return out


# revision 13
# speedup vs baseline: 2.4707x; 2.4707x over previous
"""Trainium2 Bass kernel for nn_CustomConv1D_d (rank-1 dense conv1d, stride 21).

Math: out[b, t, o] = r[b, t] for all o in [0, 237), where
  r[b, t] = sum_k w[k] * sum_c x[b, 21 t + k, c],  w = softmax(p3*i + p4*i^2).

Strategy (pure data parallel over batch, 4 batches per core):
  - Per core, view x flat as [2048 groups, 4977]; partition p owns the 16
    consecutive groups [16p, 16p+16).  Input DMA in phases of 1-3 steps:
    each phase is one dma_start moving w consecutive groups per partition
    (w*19908 contiguous bytes per descriptor, 8 descriptors per SDMA
    engine -- even).
  - HWDGE splits a dma_start's descriptors into contiguous blocks of
    ceil(n/16) assigned to engines from E64 up.  SDMA engine 15 is ~20%
    slower (its AXI port also serves the dynamic-queue descriptor rings),
    so the last two steps and all output stores are issued as
    [0:120) + [120:128) pairs: the 120-wide dma spreads evenly over
    engines 0-14 and engine 15 gets nothing, shedding ~1/8 of its load.
  - DVE: per step, segmented reduce over channels -> per-tap sums skg;
    per phase one tensor_mul by tap weights + reduce over taps ->
    acc[:, s] = r values.
  - ACT broadcasts r across 237 channels into chunk buffers; output DMA
    per chunk: per-partition w*948B contiguous descriptors.
"""

import numpy as np
from contextlib import ExitStack

import concourse.bass as bass
import concourse.tile as tile
import concourse.mybir as mybir
from concourse.bass_utils import run_bass_kernel_spmd

TAPS = 21
C = 237
B = 32
L = 10752
T = 512
NCORES = 8
BPC = B // NCORES            # 4 batches per core
NG = BPC * T                 # 2048 groups per core
SPP = 16                     # groups (steps) per partition
FD = TAPS * C                # 4977 elements per group
MAXW = 3
F32 = mybir.dt.float32

# phases: (first step, width, split_e15) -- split_e15 issues the phase as
# [0:120) + [120:128) so SDMA engine 15 gets no descriptors.
PHASES = [
    (0, 1, False),
    (1, 2, False),
    (3, 3, False),
    (6, 3, False),
    (9, 3, False),
    (12, 2, False),
    (14, 1, True),
    (15, 1, True),
]
# output chunks: (step_lo, step_hi)
OCHUNKS = [(0, 6), (6, 12), (12, 15), (15, 16)]
OMAXW = 6


class _TileContext(tile.TileContext):
    """TileContext with a post-scheduling pass that splits instructions
    carrying >1 sem wait onto preceding single-wait nops on the same
    engine -- the pinned neuronxcc rejects instructions with multiple
    sync wait commands."""

    def schedule_and_allocate(self):
        ret = super().schedule_and_allocate()
        self._split_multi_waits()
        return ret

    def _split_multi_waits(self):
        nc = self.nc
        for fn in nc.m.functions:
            for bb in fn.blocks:
                if not any(
                    inst.sync_info
                    and inst.sync_info.on_wait
                    and len(inst.sync_info.on_wait) > 1
                    for inst in bb.instructions
                ):
                    continue
                new_insts = []
                for inst in bb.instructions:
                    si = inst.sync_info
                    waits = list(si.on_wait) if si and si.on_wait else []
                    if len(waits) > 1:
                        si.on_wait = waits[-1:]
                        for w in waits[:-1]:
                            nop = mybir.InstNoOp(
                                name=f"I-splitw-{nc.next_id()}",
                                engine=inst.engine,
                                sync_info=mybir.SyncInfo(on_wait=[w], on_update=[]),
                            )
                            nc.register_instruction(nop, overwrite=True)
                            new_insts.append(nop)
                    new_insts.append(inst)
                bb.instructions[:] = new_insts


def _view(t3, p_lo, p_hi, s, w, fsz):
    """AP [p_hi-p_lo, w*fsz]: partition p rows 16p+s..16p+s+w; t3 is the
    full tensor pre-rearranged to [128, 16*fsz]."""
    return t3[p_lo:p_hi, s * fsz : (s + w) * fsz]


def _build():
    nc = bass.Bass("TRN2", target_bir_lowering=False, debug=False)
    x = (
        nc.dram_tensor("x", [NG, FD], F32, kind="ExternalInput")
        .ap()
        .rearrange("(a g) f -> a (g f)", g=SPP)
    )
    wv = nc.dram_tensor("wv", [SPP * TAPS], F32, kind="ExternalInput").ap()
    y = (
        nc.dram_tensor("y", [NG, C], F32, kind="ExternalOutput")
        .ap()
        .rearrange("(a g) c -> a (g c)", g=SPP)
    )

    with _TileContext(nc) as tc:
        with ExitStack() as ctx:
            xin = ctx.enter_context(tc.tile_pool(name="xin", bufs=2))
            sp = ctx.enter_context(tc.tile_pool(name="sp", bufs=1))
            op = ctx.enter_context(tc.tile_pool(name="op", bufs=2))

            wrep = sp.tile([128, SPP * TAPS], F32)
            nc.gpsimd.dma_start(wrep[:], wv[None, :].broadcast_to([128, SPP * TAPS]))
            skg = sp.tile([128, SPP * TAPS], F32)
            skw = sp.tile([128, MAXW * TAPS], F32)
            acc = sp.tile([128, SPP], F32)

            steps_done = set()
            emitted = set()

            for (s0, w, split) in PHASES:
                xt = xin.tile([128, MAXW * FD], F32, tag="xt")
                ranges = [(0, 120), (120, 128)] if split else [(0, 128)]
                if s0 == 15:
                    # tail: split taps so the final reduce is short
                    for (a, b) in ranges:
                        src = _view(x, a, b, s0, 1, FD)
                        for k0, tk in ((0, 11), (11, 10)):
                            nc.sync.dma_start(
                                xt[a:b, k0 * C : (k0 + tk) * C],
                                src[:, k0 * C : (k0 + tk) * C],
                            )
                    for k0, tk in ((0, 11), (11, 10)):
                        nc.vector.reduce_sum(
                            skg[:, s0 * TAPS + k0 : s0 * TAPS + k0 + tk],
                            xt[:, k0 * C : (k0 + tk) * C].rearrange(
                                "p (k c) -> p k c", c=C
                            ),
                            axis=mybir.AxisListType.X,
                        )
                else:
                    for (a, b) in ranges:
                        nc.sync.dma_start(
                            xt[a:b, : w * FD], _view(x, a, b, s0, w, FD)
                        )
                    for j in range(w):
                        nc.vector.reduce_sum(
                            skg[:, (s0 + j) * TAPS : (s0 + j + 1) * TAPS],
                            xt[:, j * FD : (j + 1) * FD].rearrange(
                                "p (k c) -> p k c", c=C
                            ),
                            axis=mybir.AxisListType.X,
                        )
                nc.vector.tensor_mul(
                    skw[:, : w * TAPS],
                    skg[:, s0 * TAPS : (s0 + w) * TAPS],
                    wrep[:, s0 * TAPS : (s0 + w) * TAPS],
                )
                nc.vector.reduce_sum(
                    acc[:, s0 : s0 + w],
                    skw[:, : w * TAPS].rearrange("p (o k) -> p o k", k=TAPS),
                    axis=mybir.AxisListType.X,
                )

                steps_done.update(range(s0, s0 + w))
                for ci, (c_lo, c_hi) in enumerate(OCHUNKS):
                    if ci in emitted:
                        continue
                    if all(s in steps_done for s in range(c_lo, c_hi)):
                        emitted.add(ci)
                        cw = c_hi - c_lo
                        osb = op.tile([128, OMAXW * C], F32, tag="osb")
                        for j, s in enumerate(range(c_lo, c_hi)):
                            nc.scalar.activation(
                                osb[:, j * C : (j + 1) * C],
                                acc[:, s : s + 1].broadcast_to([128, C]),
                                mybir.ActivationFunctionType.Identity,
                            )
                        for (a, b) in ((0, 120), (120, 128)):
                            nc.scalar.dma_start(
                                _view(y, a, b, c_lo, cw, C),
                                osb[a:b, : cw * C],
                            )
    return nc


_NC_CACHE = {}


def _get_nc():
    if "nc" not in _NC_CACHE:
        _NC_CACHE["nc"] = _build()
    return _NC_CACHE["nc"]


def _tap_weights(param3: float, param4: float) -> np.ndarray:
    i = np.arange(1, TAPS + 1, dtype=np.float32)
    logits = (np.float32(param3) * i + np.float32(param4) * i * i).astype(np.float32)
    e = np.exp(logits - logits.max(), dtype=np.float32)
    w = (e / e.sum()).astype(np.float32)
    return np.tile(w, SPP).astype(np.float32)  # [SPP*TAPS]


def run_with_results(inputs, **spmd_kwargs):
    x = np.ascontiguousarray(np.asarray(inputs["inputs"], dtype=np.float32))
    assert x.shape == (B, L, C), x.shape
    wv = _tap_weights(
        float(np.asarray(inputs["param3"])), float(np.asarray(inputs["param4"]))
    )
    xs = x.reshape(NCORES, NG, FD)
    in_maps = [{"x": xs[i], "wv": wv} for i in range(NCORES)]
    res = run_bass_kernel_spmd(_get_nc(), in_maps, list(range(NCORES)), **spmd_kwargs)
    out = np.stack([res.results[i]["y"] for i in range(NCORES)])
    return out.reshape(B, T, C).astype(np.float32, copy=False), res


def kernel(**inputs) -> np.ndarray:
    out, _ = run_with_results(inputs)
    return out


# revision 14
# speedup vs baseline: 2.5447x; 1.0299x over previous
"""Trainium2 Bass kernel for nn_CustomConv1D_d (rank-1 dense conv1d, stride 21).

Math: out[b, t, o] = r[b, t] for all o in [0, 237), where
  r[b, t] = sum_k w[k] * sum_c x[b, 21 t + k, c],  w = softmax(p3*i + p4*i^2).

Strategy (pure data parallel over batch, 4 batches per core):
  - Per core, view x flat as [2048 groups, 4977]; partition p owns the 16
    consecutive groups [16p, 16p+16).  One dma_start per step s loads
    group 16p+s for every partition: 19908B contiguous per descriptor,
    8 descriptors per SDMA engine (even).  A 6-deep tile ring keeps the
    DMA queue ahead of the consumer.
  - HWDGE deals a dma_start's descriptors to engines in contiguous blocks
    of ceil(n/16) starting at engine 0.  SDMA engine 15 is ~20% slower
    (its AXI port also serves the dynamic-queue descriptor rings), so
    steps 13-15 are issued as [0:120) + [120:128) pairs: the 120-wide dma
    spreads evenly over engines 0-14 and engine 15 gets nothing, shedding
    3/16 of its load.  Output stores stay full-width (engine 15 has the
    headroom to absorb its share).
  - DVE: per step, one segmented reduce over channels -> per-tap sums
    skg; after every few steps one tensor_mul by tap weights + reduce
    over taps -> acc[:, s] = r values.
  - ACT broadcasts r across 237 channels into chunk buffers; output DMA
    per chunk writes per-partition w*948B contiguous descriptors.
"""

import numpy as np
from contextlib import ExitStack

import concourse.bass as bass
import concourse.tile as tile
import concourse.mybir as mybir
from concourse.bass_utils import run_bass_kernel_spmd

TAPS = 21
C = 237
B = 32
L = 10752
T = 512
NCORES = 8
BPC = B // NCORES            # 4 batches per core
NG = BPC * T                 # 2048 groups per core
SPP = 16                     # groups (steps) per partition
FD = TAPS * C                # 4977 elements per group
F32 = mybir.dt.float32

SPLIT_STEPS = {13, 14, 15}   # issued as [0:120) + [120:128)
COMBINE_AT = {3: 0, 7: 4, 11: 8, 14: 12, 15: 15}  # step -> first acc col
OCHUNKS = [(0, 4), (4, 8), (8, 12), (12, 15), (15, 16)]
OMAXW = 4


class _TileContext(tile.TileContext):
    """TileContext with a post-scheduling pass that splits instructions
    carrying >1 sem wait onto preceding single-wait nops on the same
    engine -- the pinned neuronxcc rejects instructions with multiple
    sync wait commands."""

    def schedule_and_allocate(self):
        ret = super().schedule_and_allocate()
        self._split_multi_waits()
        return ret

    def _split_multi_waits(self):
        nc = self.nc
        for fn in nc.m.functions:
            for bb in fn.blocks:
                if not any(
                    inst.sync_info
                    and inst.sync_info.on_wait
                    and len(inst.sync_info.on_wait) > 1
                    for inst in bb.instructions
                ):
                    continue
                new_insts = []
                for inst in bb.instructions:
                    si = inst.sync_info
                    waits = list(si.on_wait) if si and si.on_wait else []
                    if len(waits) > 1:
                        si.on_wait = waits[-1:]
                        for w in waits[:-1]:
                            nop = mybir.InstNoOp(
                                name=f"I-splitw-{nc.next_id()}",
                                engine=inst.engine,
                                sync_info=mybir.SyncInfo(on_wait=[w], on_update=[]),
                            )
                            nc.register_instruction(nop, overwrite=True)
                            new_insts.append(nop)
                    new_insts.append(inst)
                bb.instructions[:] = new_insts


def _build():
    nc = bass.Bass("TRN2", target_bir_lowering=False, debug=False)
    x = (
        nc.dram_tensor("x", [NG, FD], F32, kind="ExternalInput")
        .ap()
        .rearrange("(a g) f -> a (g f)", g=SPP)       # [128, 16*4977]
    )
    wv = nc.dram_tensor("wv", [SPP * TAPS], F32, kind="ExternalInput").ap()
    y = (
        nc.dram_tensor("y", [NG, C], F32, kind="ExternalOutput")
        .ap()
        .rearrange("(a g) c -> a (g c)", g=SPP)       # [128, 16*237]
    )

    with _TileContext(nc) as tc:
        with ExitStack() as ctx:
            xin = ctx.enter_context(tc.tile_pool(name="xin", bufs=6))
            sp = ctx.enter_context(tc.tile_pool(name="sp", bufs=1))
            op = ctx.enter_context(tc.tile_pool(name="op", bufs=2))

            wrep = sp.tile([128, SPP * TAPS], F32)
            nc.gpsimd.dma_start(wrep[:], wv[None, :].broadcast_to([128, SPP * TAPS]))
            skg = sp.tile([128, SPP * TAPS], F32)
            skw = sp.tile([128, OMAXW * TAPS], F32)
            acc = sp.tile([128, SPP], F32)

            emitted = set()

            for s in range(SPP):
                xt = xin.tile([128, FD], F32, tag="xt")
                ranges = ((0, 120), (120, 128)) if s in SPLIT_STEPS else ((0, 128),)
                splits = ((0, 11), (11, 10)) if s == 15 else ((0, TAPS),)
                for (a, b) in ranges:
                    for k0, tk in splits:
                        nc.sync.dma_start(
                            xt[a:b, k0 * C : (k0 + tk) * C],
                            x[a:b, s * FD + k0 * C : s * FD + (k0 + tk) * C],
                        )
                for k0, tk in splits:
                    nc.vector.reduce_sum(
                        skg[:, s * TAPS + k0 : s * TAPS + k0 + tk],
                        xt[:, k0 * C : (k0 + tk) * C].rearrange(
                            "p (k c) -> p k c", c=C
                        ),
                        axis=mybir.AxisListType.X,
                    )
                if s not in COMBINE_AT:
                    continue
                c0 = COMBINE_AT[s]
                cw = s + 1 - c0
                nc.vector.tensor_mul(
                    skw[:, : cw * TAPS],
                    skg[:, c0 * TAPS : (s + 1) * TAPS],
                    wrep[:, c0 * TAPS : (s + 1) * TAPS],
                )
                nc.vector.reduce_sum(
                    acc[:, c0 : s + 1],
                    skw[:, : cw * TAPS].rearrange("p (o k) -> p o k", k=TAPS),
                    axis=mybir.AxisListType.X,
                )
                for ci, (o_lo, o_hi) in enumerate(OCHUNKS):
                    if ci in emitted or o_hi > s + 1:
                        continue
                    emitted.add(ci)
                    ow = o_hi - o_lo
                    osb = op.tile([128, OMAXW * C], F32, tag="osb")
                    for j, os_ in enumerate(range(o_lo, o_hi)):
                        nc.scalar.activation(
                            osb[:, j * C : (j + 1) * C],
                            acc[:, os_ : os_ + 1].broadcast_to([128, C]),
                            mybir.ActivationFunctionType.Identity,
                        )
                    nc.scalar.dma_start(
                        y[:, o_lo * C : o_hi * C], osb[:, : ow * C]
                    )
    return nc


_NC_CACHE = {}


def _get_nc():
    if "nc" not in _NC_CACHE:
        _NC_CACHE["nc"] = _build()
    return _NC_CACHE["nc"]


def _tap_weights(param3: float, param4: float) -> np.ndarray:
    i = np.arange(1, TAPS + 1, dtype=np.float32)
    logits = (np.float32(param3) * i + np.float32(param4) * i * i).astype(np.float32)
    e = np.exp(logits - logits.max(), dtype=np.float32)
    w = (e / e.sum()).astype(np.float32)
    return np.tile(w, SPP).astype(np.float32)  # [SPP*TAPS]


def run_with_results(inputs, **spmd_kwargs):
    x = np.ascontiguousarray(np.asarray(inputs["inputs"], dtype=np.float32))
    assert x.shape == (B, L, C), x.shape
    wv = _tap_weights(
        float(np.asarray(inputs["param3"])), float(np.asarray(inputs["param4"]))
    )
    xs = x.reshape(NCORES, NG, FD)
    in_maps = [{"x": xs[i], "wv": wv} for i in range(NCORES)]
    res = run_bass_kernel_spmd(_get_nc(), in_maps, list(range(NCORES)), **spmd_kwargs)
    out = np.stack([res.results[i]["y"] for i in range(NCORES)])
    return out.reshape(B, T, C).astype(np.float32, copy=False), res


def kernel(**inputs) -> np.ndarray:
    out, _ = run_with_results(inputs)
    return out
